# revision 19
# baseline (speedup 1.0000x reference)
"""Trainium2 Bass kernel for nn_LEIterator (CG tensor-product iterator).

Layout/sharding: 8 cores = 2 sample-halves (128 samples on SBUF partitions)
x 4 k-groups (each core computes CG combination slots k in {2g, 2g+1}).
All gather indices are compile-time constants (seeded rng), so the per-core
gathers are done host-side into tiny pre-gathered input tensors; the device
program is identical on every core (pure SPMD).

v2: everything on device is bf16 (the correctness gate is rel_err < 2e-2;
bf16 rounding costs ~4e-3), halving the output-DMA bytes to ~21.8 MB/core
(~61 us at the 358 GB/s per-core HBM limit). Compute is restructured so the
DVE runs in 4x perf mode: per nu=3 block, GpSimd builds vw = v (x) w
[128, 256] with a broadcast tensor_tensor, then the DVE expands along the
a-axis with 16 tensor_scalar_mul ops (per-partition scalar u[:, a0]),
each a dense step-1 bf16 single-src op (4x eligible). nu=2 blocks and the
vw intermediates ride on GpSimd, off the DVE critical path.
"""

import numpy as np
import ml_dtypes

import concourse.bass as bass
import concourse.mybir as mybir
from concourse.tile import TileContext
from concourse.vector_clock import ScopedClock
from concourse.bass_utils import run_bass_kernel_spmd

BF16 = ml_dtypes.bfloat16


class _SplitDrainTC(TileContext):
    """TileContext whose kernel-tail drain spreads its semaphore waits over
    single-wait NOPs — this walrus codegen allows one sync wait per
    instruction (pseudo-direct DMA lowering), and the stock drain carries
    one wait per outstanding DMA lane."""

    def _drain_and_barrier(self, tick_clock, wait_clock):
        probe = self.nc.sync.nop(nofuse=True, hint="drain_waits")
        wait_clock.add_sem_waits(
            probe.ins, ScopedClock({None: tick_clock.global_clock})
        )
        si = probe.ins.sync_info
        waits = list(si.on_wait) if si is not None and si.on_wait else []
        probe.ins.sync_info = mybir.SyncInfo(on_wait=waits[:1], on_update=[])
        for w in waits[1:]:
            n = self.nc.sync.nop(nofuse=True, hint="drain_waits")
            n.ins.sync_info = mybir.SyncInfo(on_wait=[w], on_update=[])
        self.nc.sync.drain()
        self.nc.all_engine_barrier()
        popped = self.nc._tile_sem_poison_stack.pop()
        assert popped is self._sem_poison
        self.nc.clear_and_free_semaphores(list(self.sems.allocated().values()))
        self.nc.all_engine_barrier()

K = 8        # CG m-combinations kept per l_tuple
Q = 16       # radial channels
S = 256      # samples
L_MAX = 2
HALF = 128   # samples per core (S / 2 halves)
NU2_TUPLES = 6
NU3_TUPLES = 10
NU2_BLOCKS = NU2_TUPLES * 2   # per-core: 2 k-slots per tuple
NU3_BLOCKS = NU3_TUPLES * 2
QA0 = 0
QB0 = QA0 + NU2_BLOCKS * Q
PV0 = QB0 + NU2_BLOCKS * Q
PW0 = PV0 + NU3_BLOCKS * Q
INP_W = PW0 + NU3_BLOCKS * Q   # bf16 input tensor width
PU_W = NU3_BLOCKS * Q          # fp32 input tensor width (tensor_scalar
                               # scalars must be fp32)
ROWS2 = NU2_TUPLES * K * Q * Q          # 12288 rows in full output
ROWS3 = NU3_TUPLES * K * Q * Q * Q      # 327680
TOTAL_ROWS = ROWS2 + ROWS3              # 339968

# vw/nu2 tensor_tensor engine: "gpsimd" keeps them off the DVE critical
# path; "vector" is the fallback if the Q7 TT misbehaves.
TT_ENGINE = "gpsimd"

# Per-tile split of the 64 output slabs across the three elementwise-capable
# engines (measured per-slab: DVE ~0.21us at 2x, ACT ~0.45us, Q7 ~0.5us).
DVE_N = 38
ACT_N = 16
POOL_N = 64 - DVE_N - ACT_N


def _build_structure():
    """Exact replica of reference._build_structure's rng call sequence."""
    rng = np.random.default_rng(0)
    t2 = []
    for l1 in range(L_MAX + 1):
        for l2 in range(l1, L_MAX + 1):
            ip = rng.integers(0, 2 * l1 + 1, K)
            i1 = rng.integers(0, 2 * l2 + 1, K)
            mult = (rng.random(K) + 0.5).astype(np.float32)
            t2.append(((l1, l2), ip, i1, mult))
    t3 = []
    for l1 in range(L_MAX + 1):
        for l2 in range(l1, L_MAX + 1):
            for l3 in range(l2, L_MAX + 1):
                ip = rng.integers(0, K, K)
                i1 = rng.integers(0, 2 * l3 + 1, K)
                mult = (rng.random(K) + 0.5).astype(np.float32)
                t3.append(((l1, l2, l3), ip, i1, mult))
    return t2, t3


_T2, _T3 = _build_structure()
_S2MAP = {lt: (ip, i1) for lt, ip, i1, _ in _T2}

_NC = None


def _build_program():
    bf16 = mybir.dt.bfloat16
    MULT = mybir.AluOpType.mult
    nc = bass.Bass("TRN2")

    f32 = mybir.dt.float32
    inp = nc.dram_tensor("inp", [HALF, INP_W], bf16, kind="ExternalInput")
    inp32 = nc.dram_tensor("inp32", [HALF, PU_W], f32, kind="ExternalInput")
    out2 = nc.dram_tensor("out2", [HALF, NU2_BLOCKS * Q * Q], bf16, kind="ExternalOutput")
    out3 = nc.dram_tensor("out3", [HALF, NU3_BLOCKS * Q * Q * Q], bf16, kind="ExternalOutput")

    tt_eng = {"gpsimd": lambda nc: nc.gpsimd, "vector": lambda nc: nc.vector}[
        TT_ENGINE
    ](nc)

    with _SplitDrainTC(nc) as tc:
        with (
            tc.tile_pool(name="inp", bufs=1) as ipool,
            tc.tile_pool(name="vw", bufs=5) as vwpool,
            tc.tile_pool(name="big", bufs=5) as bpool,
        ):
            tinp = ipool.tile([HALF, INP_W], bf16, tag="inp")
            nc.sync.dma_start(tinp[:], inp[:])
            tpu = ipool.tile([HALF, PU_W], f32, tag="inp32")
            nc.sync.dma_start(tpu[:], inp32[:])
            tqa = tinp[:, QA0 : QA0 + NU2_BLOCKS * Q]
            tqb = tinp[:, QB0 : QB0 + NU2_BLOCKS * Q]
            tpv = tinp[:, PV0 : PV0 + NU3_BLOCKS * Q]
            tpw = tinp[:, PW0 : PW0 + NU3_BLOCKS * Q]

            # Codegen allows ONE sync wait per instruction. 1-elem copies
            # into distinct scratch columns (no WAW between them) pull
            # cross-engine waits onto each engine's vector clock ahead of
            # its slab burst, so every subsequent instruction needs at most
            # one semaphore wait.
            scrd = ipool.tile([HALF, 8], f32, tag="scrd")
            scra = ipool.tile([HALF, 16], f32, tag="scra")
            scrp = ipool.tile([HALF, 8], f32, tag="scrp")
            nc.vector.tensor_copy(scrd[:, 7:8], tpu[:, 0:1])
            nc.scalar.copy(scra[:, 15:16], tpu[:, 0:1])
            nc.gpsimd.tensor_copy(scrp[:, 7:8], tpu[:, 0:1])

            # nu=3: per mega-tile of 4 blocks, GpSimd builds vw = v (x) w
            # [p, blk, b, c]; the DVE then writes the [p, blk, a, b, c]
            # output tile via 64 per-partition-scalar multiplies (u[:, a0]),
            # each a dense bf16 step-1 single-src op -> 4x DVE mode.
            BPM = 4
            BIGW = BPM * Q * Q * Q        # 16384 cols per output tile
            NTILES = NU3_BLOCKS // BPM

            def emit_vw(m):
                vw4 = vwpool.tile([HALF, BPM * Q * Q], bf16, tag="vw")
                sl = slice(m * BPM * Q, (m + 1) * BPM * Q)
                v = (
                    tpv[:, sl]
                    .rearrange("p (c b) -> p c b", b=Q)
                    .unsqueeze(3)
                    .broadcast_to([HALF, BPM, Q, Q])
                )
                w = (
                    tpw[:, sl]
                    .rearrange("p (c w) -> p c w", w=Q)
                    .unsqueeze(2)
                    .broadcast_to([HALF, BPM, Q, Q])
                )
                tt_eng.tensor_tensor(
                    vw4.rearrange("p (c b w) -> p c b w", b=Q, w=Q), v, w, MULT
                )
                return vw4

            vw_tiles = [emit_vw(0)]
            for m in range(NTILES):
                vw4 = vw_tiles[m]
                comb = bpool.tile([HALF, BIGW], bf16, tag="big")
                # Absorb the gpsimd vw[m] RAW wait on DVE's and ACT's clocks.
                nc.vector.tensor_copy(scrd[:, m : m + 1], vw4[:, 0:1])
                nc.scalar.copy(scra[:, m : m + 1], vw4[:, 1:2])
                for j in range(BPM * Q):
                    i, a0 = divmod(j, Q)
                    b = m * BPM + i
                    dst = comb[:, j * Q * Q : (j + 1) * Q * Q]
                    src = vw4[:, i * Q * Q : (i + 1) * Q * Q]
                    scl = tpu[:, b * Q + a0 : b * Q + a0 + 1]
                    if j < DVE_N:
                        nc.vector.tensor_scalar_mul(dst, src, scl)
                    elif j < DVE_N + ACT_N:
                        nc.scalar.mul(dst, src, scl)
                    else:
                        nc.gpsimd.tensor_scalar_mul(dst, src, scl)
                if m + 1 < NTILES:
                    vw_tiles.append(emit_vw(m + 1))
                # The tile's DMA is issued from the ACT queue. Two 1-elem ACT
                # copies of the last DVE/Pool slab cells pull those engines'
                # clocks onto ACT's, so the dma_start itself carries only the
                # irreducible ACT self-wait (async SBUF read by the DMA HW).
                nc.scalar.copy(
                    scra[:, 5 + m : 6 + m],
                    comb[:, (DVE_N - 1) * Q * Q : (DVE_N - 1) * Q * Q + 1],
                )
                nc.scalar.copy(
                    scra[:, 10 + m : 11 + m],
                    comb[:, (BPM * Q - 1) * Q * Q : (BPM * Q - 1) * Q * Q + 1],
                )
                nc.scalar.dma_start(out3[:, m * BIGW : (m + 1) * BIGW], comb[:])

            # nu=2 blocks: 3 TTs of 4 blocks into one staging tile, one DMA
            t2s = ipool.tile([HALF, NU2_BLOCKS * Q * Q], bf16, tag="o2")
            for b in range(0, NU2_BLOCKS, 4):
                sl = slice(b * Q, (b + 4) * Q)
                a = (
                    tqa[:, sl]
                    .rearrange("p (c a) -> p c a", a=Q)
                    .unsqueeze(3)
                    .broadcast_to([HALF, 4, Q, Q])
                )
                bb = (
                    tqb[:, sl]
                    .rearrange("p (c b) -> p c b", b=Q)
                    .unsqueeze(2)
                    .broadcast_to([HALF, 4, Q, Q])
                )
                o = t2s[:, b * Q * Q : (b + 4) * Q * Q].rearrange(
                    "p (c a b) -> p c a b", a=Q, b=Q
                )
                tt_eng.tensor_tensor(o, a, bb, MULT)
            nc.sync.dma_start(out2[:], t2s[:])
    return nc


def _get_nc():
    global _NC
    if _NC is None:
        _NC = _build_program()
    return _NC


def _make_in_maps(LE1):
    in_maps = []
    for c in range(8):
        h, g = divmod(c, 4)
        sl = slice(h * HALF, (h + 1) * HALF)
        buf = np.empty((HALF, INP_W), BF16)
        pu = np.empty((HALF, PU_W), np.float32)
        qa = buf[:, QA0 : QA0 + NU2_BLOCKS * Q]
        qb = buf[:, QB0 : QB0 + NU2_BLOCKS * Q]
        pv = buf[:, PV0 : PV0 + NU3_BLOCKS * Q]
        pw = buf[:, PW0 : PW0 + NU3_BLOCKS * Q]
        for ti, ((l1, l2), ip, i1, mult) in enumerate(_T2):
            for j in range(2):
                k = 2 * g + j
                b = ti * 2 + j
                qa[:, b * Q : (b + 1) * Q] = LE1[l1][ip[k], :, sl].T
                qb[:, b * Q : (b + 1) * Q] = LE1[l2][i1[k], :, sl].T * mult[k]
        for ti, ((l1, l2, l3), ip3, i13, mult3) in enumerate(_T3):
            ip2, i12 = _S2MAP[(l1, l2)]
            for j in range(2):
                k = 2 * g + j
                b = ti * 2 + j
                kk = ip3[k]
                pu[:, b * Q : (b + 1) * Q] = LE1[l1][ip2[kk], :, sl].T
                pv[:, b * Q : (b + 1) * Q] = LE1[l2][i12[kk], :, sl].T
                pw[:, b * Q : (b + 1) * Q] = LE1[l3][i13[k], :, sl].T * mult3[k]
        in_maps.append({"inp": buf, "inp32": pu})
    return in_maps


LAST_RUN = None  # BassKernelResults of the most recent kernel() call (for test.py)
TRACE = False


def kernel(LE1_l0, LE1_l1, LE1_l2):
    global LAST_RUN
    LE1 = {
        0: np.ascontiguousarray(np.asarray(LE1_l0, dtype=np.float32)),
        1: np.ascontiguousarray(np.asarray(LE1_l1, dtype=np.float32)),
        2: np.ascontiguousarray(np.asarray(LE1_l2, dtype=np.float32)),
    }
    nc = _get_nc()
    in_maps = _make_in_maps(LE1)
    LAST_RUN = run_bass_kernel_spmd(
        nc, in_maps, core_ids=list(range(8)), trace=TRACE
    )
    res = LAST_RUN.results

    out = np.empty((TOTAL_ROWS, S), np.float32)
    for c in range(8):
        h, g = divmod(c, 4)
        cs = slice(h * HALF, (h + 1) * HALF)
        o2 = res[c]["out2"].astype(np.float32)
        o3 = res[c]["out3"].astype(np.float32)
        for ti in range(NU2_TUPLES):
            for j in range(2):
                k = 2 * g + j
                b = ti * 2 + j
                r0 = ti * (K * Q * Q) + k * Q * Q
                out[r0 : r0 + Q * Q, cs] = o2[:, b * Q * Q : (b + 1) * Q * Q].T
        for ti in range(NU3_TUPLES):
            for j in range(2):
                k = 2 * g + j
                b = ti * 2 + j
                w = Q * Q * Q
                r0 = ROWS2 + ti * (K * w) + k * w
                out[r0 : r0 + w, cs] = o3[:, b * w : (b + 1) * w].T
    return out


# revision 23
# speedup vs baseline: 2.9212x; 2.9212x over previous
"""Trainium2 Bass kernel for nn_LEIterator (CG tensor-product iterator).

Layout/sharding: 8 cores = 2 sample-halves (128 samples on SBUF partitions)
x 4 k-groups (each core computes CG combination slots k in {2g, 2g+1}).
All gather indices are compile-time constants (seeded rng), so the per-core
gathers are done host-side into tiny pre-gathered input tensors; the device
program is identical on every core (pure SPMD).

v2: everything on device is bf16 (the correctness gate is rel_err < 2e-2;
bf16 rounding costs ~4e-3), halving the output-DMA bytes to ~21.8 MB/core
(~61 us at the 358 GB/s per-core HBM limit). Compute is restructured so the
DVE runs in 4x perf mode: per nu=3 block, GpSimd builds vw = v (x) w
[128, 256] with a broadcast tensor_tensor, then the DVE expands along the
a-axis with 16 tensor_scalar_mul ops (per-partition scalar u[:, a0]),
each a dense step-1 bf16 single-src op (4x eligible). nu=2 blocks and the
vw intermediates ride on GpSimd, off the DVE critical path.
"""

import numpy as np
import ml_dtypes

import concourse.bass as bass
import concourse.mybir as mybir
from concourse.tile import TileContext
from concourse.vector_clock import ScopedClock
from concourse.bass_utils import run_bass_kernel_spmd

BF16 = ml_dtypes.bfloat16


class _SplitDrainTC(TileContext):
    """TileContext whose kernel-tail drain spreads its semaphore waits over
    single-wait NOPs — this walrus codegen allows one sync wait per
    instruction (pseudo-direct DMA lowering), and the stock drain carries
    one wait per outstanding DMA lane."""

    def _drain_and_barrier(self, tick_clock, wait_clock):
        probe = self.nc.sync.nop(nofuse=True, hint="drain_waits")
        wait_clock.add_sem_waits(
            probe.ins, ScopedClock({None: tick_clock.global_clock})
        )
        si = probe.ins.sync_info
        waits = list(si.on_wait) if si is not None and si.on_wait else []
        probe.ins.sync_info = mybir.SyncInfo(on_wait=waits[:1], on_update=[])
        for w in waits[1:]:
            n = self.nc.sync.nop(nofuse=True, hint="drain_waits")
            n.ins.sync_info = mybir.SyncInfo(on_wait=[w], on_update=[])
        self.nc.sync.drain()
        self.nc.all_engine_barrier()
        popped = self.nc._tile_sem_poison_stack.pop()
        assert popped is self._sem_poison
        self.nc.clear_and_free_semaphores(list(self.sems.allocated().values()))
        self.nc.all_engine_barrier()

K = 8        # CG m-combinations kept per l_tuple
Q = 16       # radial channels
S = 256      # samples
L_MAX = 2
HALF = 128   # samples per core (S / 2 halves)
NU2_TUPLES = 6
NU3_TUPLES = 10
NU2_BLOCKS = NU2_TUPLES * 2   # per-core: 2 k-slots per tuple
NU3_BLOCKS = NU3_TUPLES * 2
QA0 = 0
QB0 = QA0 + NU2_BLOCKS * Q
PV0 = QB0 + NU2_BLOCKS * Q
PW0 = PV0 + NU3_BLOCKS * Q
INP_W = PW0 + NU3_BLOCKS * Q   # bf16 input tensor width
PU_W = NU3_BLOCKS * Q          # fp32 input tensor width (tensor_scalar
                               # scalars must be fp32)
ROWS2 = NU2_TUPLES * K * Q * Q          # 12288 rows in full output
ROWS3 = NU3_TUPLES * K * Q * Q * Q      # 327680
TOTAL_ROWS = ROWS2 + ROWS3              # 339968

# vw/nu2 tensor_tensor engine: "gpsimd" keeps them off the DVE critical
# path; "vector" is the fallback if the Q7 TT misbehaves.
TT_ENGINE = "gpsimd"

# Per-tile split of the 64 output slabs between DVE and ACT (measured
# per-slab: DVE ~0.21us at 2x mode, ACT ~0.45us). The Q7 tensor_scalar
# measured 3.9us/slab AND its SBUF-port contention stalls the DVE, so
# gpsimd gets no slabs — it only builds the vw / nu=2 tensor products.
DVE_N = 46
ACT_N = 64 - DVE_N


def _build_structure():
    """Exact replica of reference._build_structure's rng call sequence."""
    rng = np.random.default_rng(0)
    t2 = []
    for l1 in range(L_MAX + 1):
        for l2 in range(l1, L_MAX + 1):
            ip = rng.integers(0, 2 * l1 + 1, K)
            i1 = rng.integers(0, 2 * l2 + 1, K)
            mult = (rng.random(K) + 0.5).astype(np.float32)
            t2.append(((l1, l2), ip, i1, mult))
    t3 = []
    for l1 in range(L_MAX + 1):
        for l2 in range(l1, L_MAX + 1):
            for l3 in range(l2, L_MAX + 1):
                ip = rng.integers(0, K, K)
                i1 = rng.integers(0, 2 * l3 + 1, K)
                mult = (rng.random(K) + 0.5).astype(np.float32)
                t3.append(((l1, l2, l3), ip, i1, mult))
    return t2, t3


_T2, _T3 = _build_structure()
_S2MAP = {lt: (ip, i1) for lt, ip, i1, _ in _T2}

_NC = None


def _build_program():
    bf16 = mybir.dt.bfloat16
    MULT = mybir.AluOpType.mult
    nc = bass.Bass("TRN2")

    f32 = mybir.dt.float32
    inp = nc.dram_tensor("inp", [HALF, INP_W], bf16, kind="ExternalInput")
    inp32 = nc.dram_tensor("inp32", [HALF, PU_W], f32, kind="ExternalInput")
    out2 = nc.dram_tensor("out2", [HALF, NU2_BLOCKS * Q * Q], bf16, kind="ExternalOutput")
    out3 = nc.dram_tensor("out3", [HALF, NU3_BLOCKS * Q * Q * Q], bf16, kind="ExternalOutput")

    tt_eng = {"gpsimd": lambda nc: nc.gpsimd, "vector": lambda nc: nc.vector}[
        TT_ENGINE
    ](nc)

    with _SplitDrainTC(nc) as tc:
        with (
            tc.tile_pool(name="inp", bufs=1) as ipool,
            tc.tile_pool(name="vw", bufs=5) as vwpool,
            tc.tile_pool(name="big", bufs=5) as bpool,
        ):
            tinp = ipool.tile([HALF, INP_W], bf16, tag="inp")
            nc.sync.dma_start(tinp[:], inp[:])
            tpu = ipool.tile([HALF, PU_W], f32, tag="inp32")
            nc.sync.dma_start(tpu[:], inp32[:])
            tqa = tinp[:, QA0 : QA0 + NU2_BLOCKS * Q]
            tqb = tinp[:, QB0 : QB0 + NU2_BLOCKS * Q]
            tpv = tinp[:, PV0 : PV0 + NU3_BLOCKS * Q]
            tpw = tinp[:, PW0 : PW0 + NU3_BLOCKS * Q]

            # Codegen allows ONE sync wait per instruction. 1-elem copies
            # into distinct scratch columns (no WAW between them) pull
            # cross-engine waits onto each engine's vector clock ahead of
            # its slab burst, so every subsequent instruction needs at most
            # one semaphore wait.
            scrd = ipool.tile([HALF, 8], f32, tag="scrd")
            scra = ipool.tile([HALF, 16], f32, tag="scra")
            nc.vector.tensor_copy(scrd[:, 7:8], tpu[:, 0:1])
            nc.scalar.copy(scra[:, 15:16], tpu[:, 0:1])

            # nu=3: per mega-tile of 4 blocks, GpSimd builds vw = v (x) w
            # [p, blk, b, c]; the DVE then writes the [p, blk, a, b, c]
            # output tile via 64 per-partition-scalar multiplies (u[:, a0]),
            # each a dense bf16 step-1 single-src op -> 4x DVE mode.
            BPM = 4
            BIGW = BPM * Q * Q * Q        # 16384 cols per output tile
            NTILES = NU3_BLOCKS // BPM

            def emit_vw(m):
                vw4 = vwpool.tile([HALF, BPM * Q * Q], bf16, tag="vw")
                sl = slice(m * BPM * Q, (m + 1) * BPM * Q)
                v = (
                    tpv[:, sl]
                    .rearrange("p (c b) -> p c b", b=Q)
                    .unsqueeze(3)
                    .broadcast_to([HALF, BPM, Q, Q])
                )
                w = (
                    tpw[:, sl]
                    .rearrange("p (c w) -> p c w", w=Q)
                    .unsqueeze(2)
                    .broadcast_to([HALF, BPM, Q, Q])
                )
                tt_eng.tensor_tensor(
                    vw4.rearrange("p (c b w) -> p c b w", b=Q, w=Q), v, w, MULT
                )
                return vw4

            vw_tiles = [emit_vw(0)]
            for m in range(NTILES):
                vw4 = vw_tiles[m]
                comb = bpool.tile([HALF, BIGW], bf16, tag="big")
                # Absorb the gpsimd vw[m] RAW wait on DVE's and ACT's clocks.
                nc.vector.tensor_copy(scrd[:, m : m + 1], vw4[:, 0:1])
                nc.scalar.copy(scra[:, m : m + 1], vw4[:, 1:2])
                for j in range(BPM * Q):
                    i, a0 = divmod(j, Q)
                    b = m * BPM + i
                    dst = comb[:, j * Q * Q : (j + 1) * Q * Q]
                    src = vw4[:, i * Q * Q : (i + 1) * Q * Q]
                    scl = tpu[:, b * Q + a0 : b * Q + a0 + 1]
                    if j < DVE_N:
                        nc.vector.tensor_scalar_mul(dst, src, scl)
                    else:
                        nc.scalar.mul(dst, src, scl)
                if m + 1 < NTILES:
                    vw_tiles.append(emit_vw(m + 1))
                # The tile's DMA is issued from the ACT queue. A 1-elem ACT
                # copy of the last DVE slab cell pulls the DVE clock onto
                # ACT's, so the dma_start itself carries only the
                # irreducible ACT self-wait (async SBUF read by the DMA HW).
                nc.scalar.copy(
                    scra[:, 5 + m : 6 + m],
                    comb[:, (DVE_N - 1) * Q * Q : (DVE_N - 1) * Q * Q + 1],
                )
                nc.scalar.dma_start(out3[:, m * BIGW : (m + 1) * BIGW], comb[:])

            # nu=2 blocks: 3 TTs of 4 blocks into one staging tile, one DMA
            t2s = ipool.tile([HALF, NU2_BLOCKS * Q * Q], bf16, tag="o2")
            for b in range(0, NU2_BLOCKS, 4):
                sl = slice(b * Q, (b + 4) * Q)
                a = (
                    tqa[:, sl]
                    .rearrange("p (c a) -> p c a", a=Q)
                    .unsqueeze(3)
                    .broadcast_to([HALF, 4, Q, Q])
                )
                bb = (
                    tqb[:, sl]
                    .rearrange("p (c b) -> p c b", b=Q)
                    .unsqueeze(2)
                    .broadcast_to([HALF, 4, Q, Q])
                )
                o = t2s[:, b * Q * Q : (b + 4) * Q * Q].rearrange(
                    "p (c a b) -> p c a b", a=Q, b=Q
                )
                tt_eng.tensor_tensor(o, a, bb, MULT)
            nc.sync.dma_start(out2[:], t2s[:])
    return nc


def _get_nc():
    global _NC
    if _NC is None:
        _NC = _build_program()
    return _NC


def _make_in_maps(LE1):
    in_maps = []
    for c in range(8):
        h, g = divmod(c, 4)
        sl = slice(h * HALF, (h + 1) * HALF)
        buf = np.empty((HALF, INP_W), BF16)
        pu = np.empty((HALF, PU_W), np.float32)
        qa = buf[:, QA0 : QA0 + NU2_BLOCKS * Q]
        qb = buf[:, QB0 : QB0 + NU2_BLOCKS * Q]
        pv = buf[:, PV0 : PV0 + NU3_BLOCKS * Q]
        pw = buf[:, PW0 : PW0 + NU3_BLOCKS * Q]
        for ti, ((l1, l2), ip, i1, mult) in enumerate(_T2):
            for j in range(2):
                k = 2 * g + j
                b = ti * 2 + j
                qa[:, b * Q : (b + 1) * Q] = LE1[l1][ip[k], :, sl].T
                qb[:, b * Q : (b + 1) * Q] = LE1[l2][i1[k], :, sl].T * mult[k]
        for ti, ((l1, l2, l3), ip3, i13, mult3) in enumerate(_T3):
            ip2, i12 = _S2MAP[(l1, l2)]
            for j in range(2):
                k = 2 * g + j
                b = ti * 2 + j
                kk = ip3[k]
                pu[:, b * Q : (b + 1) * Q] = LE1[l1][ip2[kk], :, sl].T
                pv[:, b * Q : (b + 1) * Q] = LE1[l2][i12[kk], :, sl].T
                pw[:, b * Q : (b + 1) * Q] = LE1[l3][i13[k], :, sl].T * mult3[k]
        in_maps.append({"inp": buf, "inp32": pu})
    return in_maps


LAST_RUN = None  # BassKernelResults of the most recent kernel() call (for test.py)
TRACE = False


def kernel(LE1_l0, LE1_l1, LE1_l2):
    global LAST_RUN
    LE1 = {
        0: np.ascontiguousarray(np.asarray(LE1_l0, dtype=np.float32)),
        1: np.ascontiguousarray(np.asarray(LE1_l1, dtype=np.float32)),
        2: np.ascontiguousarray(np.asarray(LE1_l2, dtype=np.float32)),
    }
    nc = _get_nc()
    in_maps = _make_in_maps(LE1)
    LAST_RUN = run_bass_kernel_spmd(
        nc, in_maps, core_ids=list(range(8)), trace=TRACE
    )
    res = LAST_RUN.results

    out = np.empty((TOTAL_ROWS, S), np.float32)
    for c in range(8):
        h, g = divmod(c, 4)
        cs = slice(h * HALF, (h + 1) * HALF)
        o2 = res[c]["out2"].astype(np.float32)
        o3 = res[c]["out3"].astype(np.float32)
        for ti in range(NU2_TUPLES):
            for j in range(2):
                k = 2 * g + j
                b = ti * 2 + j
                r0 = ti * (K * Q * Q) + k * Q * Q
                out[r0 : r0 + Q * Q, cs] = o2[:, b * Q * Q : (b + 1) * Q * Q].T
        for ti in range(NU3_TUPLES):
            for j in range(2):
                k = 2 * g + j
                b = ti * 2 + j
                w = Q * Q * Q
                r0 = ROWS2 + ti * (K * w) + k * w
                out[r0 : r0 + w, cs] = o3[:, b * w : (b + 1) * w].T
    return out
